# revision 1
# baseline (speedup 1.0000x reference)
"""SmartLinearAppearance Trainium2 kernel.

Reference semantics (per (b, n) tracklet, reverse-time scan t = T-1 .. 0):
    xor  = (nv != 0) ^ (v_t != 0)
    prod = nv * v_t
    a_t  = prod * alpha + xor * nv          # per-part coefficient on state
    c_t  = prod * (1 - alpha) + xor * v_t   # per-part coefficient on input
    if m_t: ne = a_t[p] * ne + c_t[p] * e_t ; nv = max(nv, v_t)
    tok = where(any_t m, ne @ W.T + b, 0)

The recurrence is linear in embs given coefficients derived only from
(vis, masks), so it is reformulated as a single weighted reduction:
    ne[n, d] = sum_t w[n, t, p(d)] * embs[n, t, d]
    w = m * c * cumprod_{t' < t}(m ? a : 1),  nv = masked suffix max of vis
which streams embs from HBM exactly once (memory roofline).

Sharding: data-parallel over B across the 8 cores; the small Linear
weights are replicated (W pre-transposed on the host).
"""

import sys

sys.path.insert(0, "/opt/trn_rl_repo")

import functools

import ml_dtypes
import numpy as np

import concourse.bacc as bacc
import concourse.bass as bass
import concourse.tile as tile
from concourse import mybir
from concourse.bass_utils import run_bass_kernel_spmd

B, N, T, D, V, TOK = 8, 64, 64, 1792, 7, 512
P = 7          # parts; F = D // P = 256
F = D // P
ALPHA = float(np.float32(0.9))
ONE_MINUS_ALPHA = float(np.float32(1.0) - np.float32(0.9))
NPAIR = N // 2           # 32 tracklet pairs per core
NGRP = 8                 # embs DMA groups (8 tracklets each)
DC = D // 128            # 14 d-chunks of 128
TV = T * V               # 448

f32 = mybir.dt.float32
bf16 = mybir.dt.bfloat16


def _ap(t, offset_elems, dims):
    """Raw AP on a DRAM tensor/tile: dims = [[step, count], ...] in elements."""
    base = t[:] if hasattr(t, "shape") else t
    return bass.AP(tensor=base.tensor, offset=base.offset + offset_elems, ap=dims)


def build_nc():
    nc = bacc.Bacc()

    embs_c = nc.dram_tensor("embs_c", [N, T, D], f32, kind="ExternalInput")
    vis_c = nc.dram_tensor("vis_c", [N, TV], f32, kind="ExternalInput")
    mask_c = nc.dram_tensor("mask_c", [N, T], f32, kind="ExternalInput")
    wt_c = nc.dram_tensor("wt_c", [D, TOK], bf16, kind="ExternalInput")
    bb_c = nc.dram_tensor("bb_c", [N, TOK], f32, kind="ExternalInput")
    out_c = nc.dram_tensor("out_c", [N, TOK], f32, kind="ExternalOutput")

    with tile.TileContext(nc) as tc:
        with (
            tc.tile_pool(name="small", bufs=1) as small,
            tc.tile_pool(name="big", bufs=1) as bigp,
            tc.tile_pool(name="embs", bufs=3) as ep,
            tc.tile_pool(name="ps", bufs=1, space="PSUM") as ps,
            tc.tile_pool(name="dram", bufs=1, space="DRAM") as dram,
        ):
            # ---- constant-ish loads (issue early) ----
            wt_sb = bigp.tile([128, DC, TOK], bf16)
            nc.gpsimd.dma_start(
                out=wt_sb,
                in_=_ap(wt_c, 0, [[TOK, 128], [128 * TOK, DC], [1, TOK]]),
            )
            bb_sb = small.tile([N, TOK], f32)
            nc.sync.dma_start(out=bb_sb, in_=bb_c[:, :])

            vis = small.tile([N, TV], f32)
            nc.sync.dma_start(out=vis, in_=vis_c[:, :])
            msk = small.tile([N, T], f32)
            nc.sync.dma_start(out=msk, in_=mask_c[:, :])

            # mask broadcast view [N, T, V] (step-0 inner dim)
            mb = bass.AP(tensor=msk.tensor, offset=msk.offset,
                         ap=[msk.ap[0][:], [1, T], [0, V]])
            vis3 = vis.rearrange("n (t v) -> n t v", v=V)

            # ---- coefficient computation on [N, 448] ----
            mv = small.tile([N, T, V], f32)
            nc.vector.tensor_tensor(out=mv, in0=vis3, in1=mb,
                                    op=mybir.AluOpType.mult)
            mvf = mv.rearrange("n t v -> n (t v)")

            # exclusive masked suffix max over t (log-doubling, zero pad)
            PAD = 32 * V
            sA = small.tile([N, TV + PAD], f32)
            sB = small.tile([N, TV + PAD], f32)
            nc.vector.memset(sA, 0.0)
            nc.vector.memset(sB, 0.0)
            nc.vector.tensor_copy(out=sA[:, 0:TV - V], in_=mvf[:, V:TV])
            src, dst = sA, sB
            for k in (1, 2, 4, 8, 16, 32):
                nc.vector.tensor_tensor(
                    out=dst[:, 0:TV], in0=src[:, 0:TV],
                    in1=src[:, k * V:k * V + TV], op=mybir.AluOpType.max)
                src, dst = dst, src
            nv = src[:, 0:TV]  # exclusive suffix max, [N, 448]

            n0 = small.tile([N, TV], f32)
            nc.vector.tensor_scalar(out=n0, in0=nv, scalar1=0.0, scalar2=None,
                                    op0=mybir.AluOpType.is_gt)
            v0 = small.tile([N, TV], f32)
            nc.vector.tensor_scalar(out=v0, in0=vis, scalar1=0.0, scalar2=None,
                                    op0=mybir.AluOpType.is_gt)
            xr = small.tile([N, TV], f32)
            nc.vector.tensor_tensor(out=xr, in0=n0, in1=v0,
                                    op=mybir.AluOpType.not_equal)
            prod = small.tile([N, TV], f32)
            nc.vector.tensor_tensor(out=prod, in0=nv, in1=vis,
                                    op=mybir.AluOpType.mult)
            xnv = small.tile([N, TV], f32)
            nc.vector.tensor_tensor(out=xnv, in0=xr, in1=nv,
                                    op=mybir.AluOpType.mult)
            av = small.tile([N, TV], f32)
            nc.vector.scalar_tensor_tensor(
                out=av, in0=prod, scalar=ALPHA, in1=xnv,
                op0=mybir.AluOpType.mult, op1=mybir.AluOpType.add)
            xv = small.tile([N, TV], f32)
            nc.vector.tensor_tensor(out=xv, in0=xr, in1=vis,
                                    op=mybir.AluOpType.mult)
            cc = small.tile([N, TV], f32)
            nc.vector.scalar_tensor_tensor(
                out=cc, in0=prod, scalar=ONE_MINUS_ALPHA, in1=xv,
                op0=mybir.AluOpType.mult, op1=mybir.AluOpType.add)

            # g = m * (a - 1) + 1, staged into gbuf with a leading slot of ones
            gb = small.tile([N, TV + V], f32)
            nc.vector.memset(gb[:, 0:V], 1.0)
            av3 = av.rearrange("n (t v) -> n t v", v=V)
            gb3 = _ap(gb, V, [gb.ap[0][:], [V, T], [1, V]])
            nc.vector.scalar_tensor_tensor(
                out=gb3, in0=av3, scalar=1.0, in1=mb,
                op0=mybir.AluOpType.subtract, op1=mybir.AluOpType.mult)
            nc.vector.tensor_scalar(out=gb[:, V:V + TV], in0=gb[:, V:V + TV],
                                    scalar1=1.0, scalar2=None,
                                    op0=mybir.AluOpType.add)

            # exclusive cumulative product over t per part (scan on data0 =
            # [1, g_0, ..., g_{T-2}])
            pb = small.tile([N, TV], f32)
            for p in range(V):
                dview = _ap(gb, p, [gb.ap[0][:], [V, T]])
                oview = _ap(pb, p, [pb.ap[0][:], [V, T]])
                nc.vector.tensor_tensor_scan(
                    out=oview, data0=dview, data1=dview, initial=1.0,
                    op0=mybir.AluOpType.mult, op1=mybir.AluOpType.bypass)

            mc = small.tile([N, T, V], f32)
            nc.vector.tensor_tensor(
                out=mc, in0=cc.rearrange("n (t v) -> n t v", v=V), in1=mb,
                op=mybir.AluOpType.mult)
            wco = small.tile([N, TV], f32)
            nc.vector.tensor_tensor(out=wco, in0=mc.rearrange("n t v -> n (t v)"),
                                    in1=pb, op=mybir.AluOpType.mult)

            # nm = any(mask) per tracklet
            nm = small.tile([N, 1], f32)
            nc.vector.tensor_reduce(out=nm, in_=msk, axis=mybir.AxisListType.X,
                                    op=mybir.AluOpType.max)

            # ---- block-diagonal weights via DRAM round trip ----
            w2 = dram.tile([N, TV], f32)
            nc.sync.dma_start(out=w2, in_=wco)
            wbd = small.tile([128, NPAIR, 2, V], bf16)
            nc.vector.memset(wbd, 0.0)
            # wbd[(par, t), (i, par', p)] = delta(par, par') * w[2i+par, t, p]
            nc.gpsimd.dma_start(
                out=wbd[0:T, :, 0, :],
                in_=_ap(w2, 0, [[V, T], [2 * TV, NPAIR], [1, V]]))
            nc.gpsimd.dma_start(
                out=wbd[T:128, :, 1, :],
                in_=_ap(w2, TV, [[V, T], [2 * TV, NPAIR], [1, V]]))

            # ---- stage 1: neT[d, n] = sum_t w[n, t, p(d)] * embs[n, t, d] ----
            neT_ps = ps.tile([128, DC, N], f32)
            for g in range(NGRP):
                et = ep.tile([128, 4, D], bf16)
                nc.gpsimd.dma_start(
                    out=et[:, :, :],
                    in_=_ap(embs_c, g * 8 * T * D,
                            [[T * D, 2], [D, T], [2 * T * D, 4], [1, D]]))
                for jj in range(4):
                    ip = 4 * g + jj
                    lhs_all = et[:, jj, :]
                    for dc in range(DC):
                        nc.tensor.matmul(
                            out=neT_ps[:, dc, 2 * ip:2 * ip + 2],
                            lhsT=lhs_all[:, dc * 128:(dc + 1) * 128],
                            rhs=wbd[:, ip, :, dc // 2],
                            start=True, stop=True)
                # drain this group's columns to SBUF (with bf16 downcast)
            neT_sb = small.tile([128, DC, N], bf16)
            nc.vector.tensor_copy(out=neT_sb, in_=neT_ps)

            # ---- stage 2: tok[n, k] = sum_d neT[d, n] * wt[d, k] ----
            tok_ps = ps.tile([N, TOK], f32)
            for dc in range(DC):
                nc.tensor.matmul(
                    out=tok_ps,
                    lhsT=neT_sb[:, dc, :],
                    rhs=wt_sb[:, dc, :],
                    start=(dc == 0), stop=(dc == DC - 1))

            tok_sb = small.tile([N, TOK], f32)
            nc.vector.tensor_add(out=tok_sb, in0=tok_ps, in1=bb_sb)
            nc.vector.tensor_scalar_mul(out=tok_sb, in0=tok_sb, scalar1=nm)
            nc.sync.dma_start(out=out_c[:, :], in_=tok_sb)

    nc.compile()
    return nc


@functools.lru_cache(maxsize=1)
def _get_nc():
    return build_nc()


def _prep_in_maps(embs, vis, masks, W, b):
    wt = np.ascontiguousarray(W.T).astype(ml_dtypes.bfloat16)
    bb = np.ascontiguousarray(np.broadcast_to(
        b.astype(np.float32), (N, TOK)))
    maskf = masks.astype(np.float32)
    in_maps = []
    for c in range(B):
        in_maps.append({
            "embs_c": np.ascontiguousarray(embs[c]),
            "vis_c": np.ascontiguousarray(vis[c].reshape(N, TV)),
            "mask_c": np.ascontiguousarray(maskf[c]),
            "wt_c": wt,
            "bb_c": bb,
        })
    return in_maps


def run(embs, vis, masks, W, b, **run_kwargs):
    nc = _get_nc()
    in_maps = _prep_in_maps(embs, vis, masks, W, b)
    res = run_bass_kernel_spmd(nc, in_maps, core_ids=list(range(B)),
                               **run_kwargs)
    out = np.stack([res.results[c]["out_c"] for c in range(B)], axis=0)
    return out, res


def kernel(embs, vis, masks, W, b):
    out, _ = run(embs, vis, masks, W, b)
    return out



# revision 12
# speedup vs baseline: 1.0962x; 1.0962x over previous
"""SmartLinearAppearance Trainium2 kernel.

Reference semantics (per (b, n) tracklet, reverse-time scan t = T-1 .. 0):
    xor  = (nv != 0) ^ (v_t != 0)
    prod = nv * v_t
    a_t  = prod * alpha + xor * nv          # per-part coefficient on state
    c_t  = prod * (1 - alpha) + xor * v_t   # per-part coefficient on input
    if m_t: ne = a_t[p] * ne + c_t[p] * e_t ; nv = max(nv, v_t)
    tok = where(any_t m, ne @ W.T + b, 0)

The recurrence is linear in embs given coefficients derived only from
(vis, masks), so it is reformulated as a single weighted reduction:
    ne[n, d] = sum_t w[n, t, p(d)] * embs[n, t, d]
    w = m * c * cumprod_{t' < t}(m ? a : 1)
Masked timesteps are exact no-ops of the recurrence, so valid timesteps
are compacted on the host (ragged -> padded to the global max valid
length TP) and embs is pre-cast to bf16, shrinking the HBM stream to
TP/T * 1/2 of the naive f32 read.  The per-tracklet coefficient chain
runs on-device from the compacted (vis, masks); the block-diagonal
per-pair weight matrix is built on-chip via PE transposes (no DRAM
round trip).  Bias + final masking are folded into the stage-2 matmul
accumulation using a host-provided (any-mask) row.

Sharding: data-parallel over B across the 8 cores; the Linear weights
are replicated (pre-transposed/pre-tiled on the host).
"""

import sys

sys.path.insert(0, "/opt/trn_rl_repo")

import functools

import ml_dtypes
import numpy as np

import concourse.bacc as bacc
import concourse.bass as bass
import concourse.tile as tile
from concourse import mybir
from concourse.bass_utils import run_bass_kernel_spmd

B, N, T, D, V, TOK = 8, 64, 64, 1792, 7, 512
P = 7          # parts; F = D // P = 256
F = D // P
ALPHA = float(np.float32(0.9))
ONE_MINUS_ALPHA = float(np.float32(1.0) - np.float32(0.9))
NPAIR = N // 2           # 32 tracklet pairs per core
NGRP = 16                # embs DMA groups (2 pairs each)
DC = D // 128            # 14 d-chunks of 128

f32 = mybir.dt.float32
bf16 = mybir.dt.bfloat16


def _ap(t, offset_elems, dims):
    """Raw AP on a DRAM tensor/tile: dims = [[step, count], ...] in elements."""
    base = t[:] if hasattr(t, "shape") else t
    return bass.AP(tensor=base.tensor, offset=base.offset + offset_elems, ap=dims)


def build_nc(TP, debug=False):
    TVp = TP * V
    SH = [k for k in (1, 2, 4, 8, 16, 32) if k < TP]
    PAD = (SH[-1] if SH else 1) * V
    nc = bacc.Bacc()

    # host layout: [NGRP, 2(member), TP, 2(pair-in-group), D]
    embs_c = nc.dram_tensor("embs_c", [NGRP, 2, TP, 2, D], bf16,
                            kind="ExternalInput")
    vis_c = nc.dram_tensor("vis_c", [N, TVp], f32, kind="ExternalInput")
    mask_c = nc.dram_tensor("mask_c", [N, TP], f32, kind="ExternalInput")
    wt_c = nc.dram_tensor("wt_c", [128, DC * TOK], bf16, kind="ExternalInput")
    nmb_c = nc.dram_tensor("nmb_c", [1, N], bf16, kind="ExternalInput")
    bt_c = nc.dram_tensor("bt_c", [1, TOK], bf16, kind="ExternalInput")
    idn_c = nc.dram_tensor("idn_c", [64, 32], f32, kind="ExternalInput")
    out_c = nc.dram_tensor("out_c", [N, TOK], f32, kind="ExternalOutput")
    if debug:
        dbg_wco = nc.dram_tensor("dbg_wco", [N, TVp], f32, kind="ExternalOutput")
        dbg_wbd = nc.dram_tensor("dbg_wbd", [2 * TP, 2 * NPAIR * V], f32,
                                 kind="ExternalOutput")
        dbg_neT = nc.dram_tensor("dbg_neT", [128, DC * N], f32,
                                 kind="ExternalOutput")

    with tile.TileContext(nc) as tc:
        with (
            tc.tile_pool(name="small", bufs=1) as small,
            tc.tile_pool(name="big", bufs=1) as bigp,
            tc.tile_pool(name="embs", bufs=NGRP) as ep,
            tc.tile_pool(name="ps", bufs=1, space="PSUM") as ps,
        ):
            # ---- small loads first (sync ring) ----
            vis = small.tile([N, TVp], f32)
            nc.sync.dma_start(out=vis, in_=vis_c[:, :])
            msk = small.tile([N, TP], f32)
            nc.sync.dma_start(out=msk, in_=mask_c[:, :])
            nmb_sb = small.tile([1, N], bf16)
            nc.sync.dma_start(out=nmb_sb, in_=nmb_c[:, :])
            bt_sb = small.tile([1, TOK], bf16)
            nc.sync.dma_start(out=bt_sb, in_=bt_c[:, :])
            idn = small.tile([64, 32], f32)
            nc.sync.dma_start(out=idn, in_=idn_c[:, :])

            # ---- embs stream: NGRP groups of 2 pairs, split across the
            # gpsimd (swdge) and scalar (hwdge) rings for parallel
            # descriptor generation ----
            ets = []
            for g in range(NGRP):
                et = ep.tile([2 * TP, 2, D], bf16)
                eng = nc.gpsimd if g % 2 == 0 else nc.scalar
                eng.dma_start(
                    out=et[:, :, :],
                    in_=_ap(embs_c, g * 2 * TP * 2 * D,
                            [[TP * 2 * D, 2], [2 * D, TP], [1, 2 * D]]))
                ets.append(et)

            # wt after the embs groups (its data is needed last)
            wt_sb = bigp.tile([128, DC, TOK], bf16)
            nc.scalar.dma_start(
                out=wt_sb, in_=_ap(wt_c, 0, [[DC * TOK, 128], [1, DC * TOK]]))

            # mask broadcast view [N, TP, V] (step-0 inner dim)
            mb = bass.AP(tensor=msk.tensor, offset=msk.offset,
                         ap=[msk.ap[0][:], [1, TP], [0, V]])
            vis3 = vis.rearrange("n (t v) -> n t v", v=V)

            # ---- coefficient computation on [N, TP*V] ----
            mv = small.tile([N, TP, V], f32)
            nc.vector.tensor_tensor(out=mv, in0=vis3, in1=mb,
                                    op=mybir.AluOpType.mult)
            mvf = mv.rearrange("n t v -> n (t v)")

            # exclusive masked suffix max over t (log-doubling, zero pad)
            sA = small.tile([N, TVp + PAD], f32)
            sB = small.tile([N, TVp + PAD], f32)
            nc.vector.memset(sA, 0.0)
            nc.vector.memset(sB, 0.0)
            nc.vector.tensor_copy(out=sA[:, 0:TVp - V], in_=mvf[:, V:TVp])
            src, dst = sA, sB
            for k in SH:
                nc.vector.tensor_tensor(
                    out=dst[:, 0:TVp], in0=src[:, 0:TVp],
                    in1=src[:, k * V:k * V + TVp], op=mybir.AluOpType.max)
                src, dst = dst, src
            nv = src[:, 0:TVp]  # exclusive suffix max, [N, TP*V]

            n0 = small.tile([N, TVp], f32)
            nc.vector.tensor_scalar(out=n0, in0=nv, scalar1=0.0, scalar2=None,
                                    op0=mybir.AluOpType.is_gt)
            v0 = small.tile([N, TVp], f32)
            nc.vector.tensor_scalar(out=v0, in0=vis, scalar1=0.0, scalar2=None,
                                    op0=mybir.AluOpType.is_gt)
            xr = small.tile([N, TVp], f32)
            nc.vector.tensor_tensor(out=xr, in0=n0, in1=v0,
                                    op=mybir.AluOpType.not_equal)
            prod = small.tile([N, TVp], f32)
            nc.vector.tensor_tensor(out=prod, in0=nv, in1=vis,
                                    op=mybir.AluOpType.mult)
            xnv = small.tile([N, TVp], f32)
            nc.vector.tensor_tensor(out=xnv, in0=xr, in1=nv,
                                    op=mybir.AluOpType.mult)
            av = small.tile([N, TVp], f32)
            nc.vector.scalar_tensor_tensor(
                out=av, in0=prod, scalar=ALPHA, in1=xnv,
                op0=mybir.AluOpType.mult, op1=mybir.AluOpType.add)
            xv = small.tile([N, TVp], f32)
            nc.vector.tensor_tensor(out=xv, in0=xr, in1=vis,
                                    op=mybir.AluOpType.mult)
            cc = small.tile([N, TVp], f32)
            nc.vector.scalar_tensor_tensor(
                out=cc, in0=prod, scalar=ONE_MINUS_ALPHA, in1=xv,
                op0=mybir.AluOpType.mult, op1=mybir.AluOpType.add)

            # g = m * (a - 1) + 1, staged into gbuf with a leading slot of ones
            gb = small.tile([N, TVp + V], f32)
            nc.vector.memset(gb[:, 0:V], 1.0)
            av3 = av.rearrange("n (t v) -> n t v", v=V)
            gb3 = _ap(gb, V, [gb.ap[0][:], [V, TP], [1, V]])
            nc.vector.scalar_tensor_tensor(
                out=gb3, in0=av3, scalar=1.0, in1=mb,
                op0=mybir.AluOpType.subtract, op1=mybir.AluOpType.mult)
            nc.vector.tensor_scalar(out=gb[:, V:V + TVp], in0=gb[:, V:V + TVp],
                                    scalar1=1.0, scalar2=None,
                                    op0=mybir.AluOpType.add)

            # exclusive cumulative product over t per part (scan on data0 =
            # [1, g_0, ..., g_{TP-2}])
            pb = small.tile([N, TVp], f32)
            for p in range(V):
                dview = _ap(gb, p, [gb.ap[0][:], [V, TP]])
                oview = _ap(pb, p, [pb.ap[0][:], [V, TP]])
                nc.vector.tensor_tensor_scan(
                    out=oview, data0=dview, data1=dview, initial=1.0,
                    op0=mybir.AluOpType.mult, op1=mybir.AluOpType.bypass)

            mc = small.tile([N, TP, V], f32)
            nc.vector.tensor_tensor(
                out=mc, in0=cc.rearrange("n (t v) -> n t v", v=V), in1=mb,
                op=mybir.AluOpType.mult)
            wco = small.tile([N, TVp], f32)
            nc.vector.tensor_tensor(out=wco, in0=mc.rearrange("n t v -> n (t v)"),
                                    in1=pb, op=mybir.AluOpType.mult)
            wco3 = wco.rearrange("n (t v) -> n t v", v=V)

            # ---- block-diagonal weights built on-chip ----
            # vis/mask rows are host-permuted to [evens | odds]; transpose
            # each parity block per part onto PSUM (transpose outputs must
            # land at PSUM partition 0, so both parities go to base 0 in
            # separate free columns; the odd block is then partition-
            # shifted to base TP by a small SBUF->SBUF DMA).
            wT_ps = ps.tile([TP, 2, V, 32], f32)
            for i in range(2):
                for p in range(V):
                    nc.tensor.transpose(
                        out=wT_ps[:, i, p, :],
                        in_=wco3[i * 32:(i + 1) * 32, :, p],
                        identity=idn[i * 32:(i + 1) * 32, :])

            # wbd[(i, t), i', ip, p] = delta(i, i') * w[2*ip + i, t, p]
            wbd = small.tile([2 * TP, 2, NPAIR, V], bf16)
            nc.vector.memset(wbd, 0.0)
            nc.vector.tensor_copy(
                out=wbd[0:TP, 0, :, :],
                in_=wT_ps[:, 0, :, :].rearrange("t p c -> t c p"))
            sodd = small.tile([TP, NPAIR, V], bf16)
            nc.vector.tensor_copy(
                out=sodd,
                in_=wT_ps[:, 1, :, :].rearrange("t p c -> t c p"))
            nc.sync.dma_start(out=wbd[TP:2 * TP, 1, :, :], in_=sodd[:, :, :])

            # ---- stage 1: neT[d, n] = sum_t w[n, t, p(d)] * embs[n, t, d] ----
            neT_ps = ps.tile([128, DC, N], f32)
            neT_sb = bigp.tile([128, DC, N], bf16)
            for g in range(NGRP):
                et = ets[g]
                for jj in range(2):
                    ip = 2 * g + jj
                    lhs_all = et[:, jj, :]
                    for dc in range(DC):
                        nc.tensor.matmul(
                            out=neT_ps[:, dc, 2 * ip:2 * ip + 2],
                            lhsT=lhs_all[:, dc * 128:(dc + 1) * 128],
                            rhs=wbd[:, :, ip, dc // 2],
                            start=True, stop=True)
                # drain this group's columns to SBUF (bf16 downcast)
                nc.vector.tensor_copy(
                    out=neT_sb[:, :, 4 * g:4 * g + 4],
                    in_=neT_ps[:, :, 4 * g:4 * g + 4])

            # ---- stage 2: tok[n, k] = nm[n]*b[k] + sum_d neT[d, n]*wt[d, k] ----
            tok_ps = ps.tile([N, TOK], f32)
            nc.tensor.matmul(out=tok_ps, lhsT=nmb_sb, rhs=bt_sb,
                             start=True, stop=False)
            for dc in range(DC):
                nc.tensor.matmul(
                    out=tok_ps,
                    lhsT=neT_sb[:, dc, :],
                    rhs=wt_sb[:, dc, :],
                    start=False, stop=(dc == DC - 1))

            tok_sb = small.tile([N, TOK], f32)
            nc.vector.tensor_copy(out=tok_sb, in_=tok_ps)
            nc.sync.dma_start(out=out_c[:, :], in_=tok_sb)

            if debug:
                nc.sync.dma_start(out=dbg_wco[:, :], in_=wco)
                wbd_f = small.tile([2 * TP, 2 * NPAIR * V], f32)
                nc.vector.tensor_copy(
                    out=wbd_f, in_=wbd.rearrange("t a b c -> t (a b c)"))
                nc.sync.dma_start(out=dbg_wbd[:, :], in_=wbd_f)
                neT_f = small.tile([128, DC * N], f32)
                nc.vector.tensor_copy(
                    out=neT_f, in_=neT_sb.rearrange("d a b -> d (a b)"))
                nc.sync.dma_start(out=dbg_neT[:, :], in_=neT_f)

    nc.compile()
    return nc


@functools.lru_cache(maxsize=2)
def _get_nc(TP):
    return build_nc(TP)


def _prep_in_maps(embs, vis, masks, W, b):
    masks = np.asarray(masks)
    L = masks.sum(axis=2)                      # [B, N]
    TP = max(1, int(L.max()))

    # stable argsort of ~mask puts valid timesteps first, in t order
    order = np.argsort(~masks, axis=2, kind="stable")[:, :, :TP]  # [B,N,TP]

    embs_bf = np.asarray(embs).astype(ml_dtypes.bfloat16)
    embs_cmp = np.take_along_axis(embs_bf, order[..., None], axis=2)
    vis_cmp = np.take_along_axis(np.asarray(vis, np.float32),
                                 order[..., None], axis=2)  # [B,N,TP,V]
    mask_cmp = (np.arange(TP)[None, None, :] < L[..., None]).astype(np.float32)

    # row permutation: evens first, odds second (for parity-block transposes)
    perm = np.concatenate([np.arange(0, N, 2), np.arange(1, N, 2)])

    wt2 = np.ascontiguousarray(
        W.T.astype(ml_dtypes.bfloat16).reshape(DC, 128, TOK)
        .transpose(1, 0, 2).reshape(128, DC * TOK))
    bt = np.ascontiguousarray(b.astype(ml_dtypes.bfloat16)[None, :])
    idn = np.concatenate([np.eye(32, dtype=np.float32)] * 2, axis=0)

    in_maps = []
    for c in range(B):
        # [N, TP, D] -> [NGRP, 2(j), 2(i), TP, D] -> [NGRP, 2(i), TP, 2(j), D]
        eg = np.ascontiguousarray(
            embs_cmp[c].reshape(NGRP, 2, 2, TP, D).transpose(0, 2, 3, 1, 4))
        nmb = (L[c] > 0).astype(ml_dtypes.bfloat16)[None, :]
        in_maps.append({
            "embs_c": eg,
            "vis_c": np.ascontiguousarray(vis_cmp[c][perm].reshape(N, TP * V)),
            "mask_c": np.ascontiguousarray(mask_cmp[c][perm]),
            "wt_c": wt2,
            "nmb_c": np.ascontiguousarray(nmb),
            "bt_c": bt,
            "idn_c": idn,
        })
    return TP, in_maps


def run(embs, vis, masks, W, b, **run_kwargs):
    TP, in_maps = _prep_in_maps(embs, vis, masks, W, b)
    nc = _get_nc(TP)
    res = run_bass_kernel_spmd(nc, in_maps, core_ids=list(range(B)),
                               **run_kwargs)
    out = np.stack([res.results[c]["out_c"] for c in range(B)], axis=0)
    return out, res


def kernel(embs, vis, masks, W, b):
    out, _ = run(embs, vis, masks, W, b)
    return out


# revision 14
# speedup vs baseline: 1.2221x; 1.1148x over previous
"""SmartLinearAppearance Trainium2 kernel.

Reference semantics (per (b, n) tracklet, reverse-time scan t = T-1 .. 0):
    xor  = (nv != 0) ^ (v_t != 0)
    prod = nv * v_t
    a_t  = prod * alpha + xor * nv          # per-part coefficient on state
    c_t  = prod * (1 - alpha) + xor * v_t   # per-part coefficient on input
    if m_t: ne = a_t[p] * ne + c_t[p] * e_t ; nv = max(nv, v_t)
    tok = where(any_t m, ne @ W.T + b, 0)

The recurrence is linear in embs given coefficients derived only from
(vis, masks), so it is reformulated as a single weighted reduction:
    ne[n, d] = sum_t w[n, t, p(d)] * embs[n, t, d]
    w = m * c * cumprod_{t' < t}(m ? a : 1)
Masked timesteps are exact no-ops of the recurrence, so valid timesteps
are compacted on the host (ragged -> padded to the global max valid
length TP) and embs is pre-cast to bf16, shrinking the HBM stream to
TP/T * 1/2 of the naive f32 read.  The per-tracklet coefficient chain
runs on-device from the compacted (vis, masks); the block-diagonal
per-pair weight matrix is built on-chip via PE transposes (no DRAM
round trip).  Bias + final masking are folded into the stage-2 matmul
accumulation using a host-provided (any-mask) row.

Sharding: data-parallel over B across the 8 cores; the Linear weights
are replicated (pre-transposed/pre-tiled on the host).
"""

import sys

sys.path.insert(0, "/opt/trn_rl_repo")

import functools

import ml_dtypes
import numpy as np

import concourse.bacc as bacc
import concourse.bass as bass
import concourse.tile as tile
from concourse import mybir
from concourse.bass_utils import run_bass_kernel_spmd

B, N, T, D, V, TOK = 8, 64, 64, 1792, 7, 512
P = 7          # parts; F = D // P = 256
F = D // P
ALPHA = float(np.float32(0.9))
ONE_MINUS_ALPHA = float(np.float32(1.0) - np.float32(0.9))
NPAIR = N // 2           # 32 tracklet pairs per core
NGRP = 16                # embs DMA groups (2 pairs each)
DC = D // 128            # 14 d-chunks of 128

f32 = mybir.dt.float32
bf16 = mybir.dt.bfloat16


def _ap(t, offset_elems, dims):
    """Raw AP on a DRAM tensor/tile: dims = [[step, count], ...] in elements."""
    base = t[:] if hasattr(t, "shape") else t
    return bass.AP(tensor=base.tensor, offset=base.offset + offset_elems, ap=dims)


def build_nc(TP, debug=False):
    TVp = TP * V
    SH = [k for k in (1, 2, 4, 8, 16, 32) if k < TP]
    PAD = (SH[-1] if SH else 1) * V
    nc = bacc.Bacc()

    # host layout: [NGRP, 2(member), TP, 2(pair-in-group), D]
    embs_c = nc.dram_tensor("embs_c", [NGRP, 2, TP, 2, D], bf16,
                            kind="ExternalInput")
    vis_c = nc.dram_tensor("vis_c", [N, TVp], f32, kind="ExternalInput")
    mask_c = nc.dram_tensor("mask_c", [N, TP], f32, kind="ExternalInput")
    wt_c = nc.dram_tensor("wt_c", [128, DC * TOK], bf16, kind="ExternalInput")
    nmb_c = nc.dram_tensor("nmb_c", [1, N], bf16, kind="ExternalInput")
    bt_c = nc.dram_tensor("bt_c", [1, TOK], bf16, kind="ExternalInput")
    idn_c = nc.dram_tensor("idn_c", [64, 32], f32, kind="ExternalInput")
    out_c = nc.dram_tensor("out_c", [N, TOK], f32, kind="ExternalOutput")
    if debug:
        dbg_wco = nc.dram_tensor("dbg_wco", [N, TVp], f32, kind="ExternalOutput")
        dbg_wbd = nc.dram_tensor("dbg_wbd", [2 * TP, 2 * NPAIR * V], f32,
                                 kind="ExternalOutput")
        dbg_neT = nc.dram_tensor("dbg_neT", [128, DC * N], f32,
                                 kind="ExternalOutput")

    with tile.TileContext(nc) as tc:
        with (
            tc.tile_pool(name="small", bufs=1) as small,
            tc.tile_pool(name="big", bufs=1) as bigp,
            tc.tile_pool(name="embs", bufs=NGRP) as ep,
            tc.tile_pool(name="ps", bufs=1, space="PSUM") as ps,
            tc.tile_pool(name="pst", bufs=2, space="PSUM") as pst,
        ):
            # ---- small loads first so their descriptors beat the embs
            # flood into the DMA queues (gpsimd ring goes first) ----
            vis = small.tile([N, TVp], f32)
            nc.gpsimd.dma_start(out=vis, in_=vis_c[:, :])
            msk = small.tile([N, TP], f32)
            nc.gpsimd.dma_start(out=msk, in_=mask_c[:, :])
            nmb_sb = small.tile([1, N], bf16)
            nc.sync.dma_start(out=nmb_sb, in_=nmb_c[:, :])
            bt_sb = small.tile([1, TOK], bf16)
            nc.sync.dma_start(out=bt_sb, in_=bt_c[:, :])
            idn = small.tile([64, 32], f32)
            nc.sync.dma_start(out=idn, in_=idn_c[:, :])

            # ---- embs stream: NGRP groups of 2 pairs, split across the
            # gpsimd (swdge) and scalar (hwdge) rings for parallel
            # descriptor generation ----
            ets = []
            for g in range(NGRP):
                et = ep.tile([2 * TP, 2, D], bf16)
                eng = nc.gpsimd if g % 2 == 0 else nc.scalar
                eng.dma_start(
                    out=et[:, :, :],
                    in_=_ap(embs_c, g * 2 * TP * 2 * D,
                            [[TP * 2 * D, 2], [2 * D, TP], [1, 2 * D]]))
                ets.append(et)

            # wt after the embs groups (its data is needed last)
            wt_sb = bigp.tile([128, DC, TOK], bf16)
            nc.scalar.dma_start(
                out=wt_sb, in_=_ap(wt_c, 0, [[DC * TOK, 128], [1, DC * TOK]]))

            # mask broadcast view [N, TP, V] (step-0 inner dim)
            mb = bass.AP(tensor=msk.tensor, offset=msk.offset,
                         ap=[msk.ap[0][:], [1, TP], [0, V]])
            vis3 = vis.rearrange("n (t v) -> n t v", v=V)

            # ---- coefficient computation on [N, TP*V] ----
            mv = small.tile([N, TP, V], f32)
            nc.vector.tensor_tensor(out=mv, in0=vis3, in1=mb,
                                    op=mybir.AluOpType.mult)
            mvf = mv.rearrange("n t v -> n (t v)")

            # exclusive masked suffix max over t (log-doubling, zero pad)
            sA = small.tile([N, TVp + PAD], f32)
            sB = small.tile([N, TVp + PAD], f32)
            nc.vector.memset(sA, 0.0)
            nc.vector.memset(sB, 0.0)
            nc.vector.tensor_copy(out=sA[:, 0:TVp - V], in_=mvf[:, V:TVp])
            src, dst = sA, sB
            for k in SH:
                nc.vector.tensor_tensor(
                    out=dst[:, 0:TVp], in0=src[:, 0:TVp],
                    in1=src[:, k * V:k * V + TVp], op=mybir.AluOpType.max)
                src, dst = dst, src
            nv = src[:, 0:TVp]  # exclusive suffix max, [N, TP*V]

            n0 = small.tile([N, TVp], f32)
            nc.vector.tensor_scalar(out=n0, in0=nv, scalar1=0.0, scalar2=None,
                                    op0=mybir.AluOpType.is_gt)
            v0 = small.tile([N, TVp], f32)
            nc.vector.tensor_scalar(out=v0, in0=vis, scalar1=0.0, scalar2=None,
                                    op0=mybir.AluOpType.is_gt)
            xr = small.tile([N, TVp], f32)
            nc.vector.tensor_tensor(out=xr, in0=n0, in1=v0,
                                    op=mybir.AluOpType.not_equal)
            prod = small.tile([N, TVp], f32)
            nc.vector.tensor_tensor(out=prod, in0=nv, in1=vis,
                                    op=mybir.AluOpType.mult)
            xnv = small.tile([N, TVp], f32)
            nc.vector.tensor_tensor(out=xnv, in0=xr, in1=nv,
                                    op=mybir.AluOpType.mult)
            av = small.tile([N, TVp], f32)
            nc.vector.scalar_tensor_tensor(
                out=av, in0=prod, scalar=ALPHA, in1=xnv,
                op0=mybir.AluOpType.mult, op1=mybir.AluOpType.add)
            xv = small.tile([N, TVp], f32)
            nc.vector.tensor_tensor(out=xv, in0=xr, in1=vis,
                                    op=mybir.AluOpType.mult)
            cc = small.tile([N, TVp], f32)
            nc.vector.scalar_tensor_tensor(
                out=cc, in0=prod, scalar=ONE_MINUS_ALPHA, in1=xv,
                op0=mybir.AluOpType.mult, op1=mybir.AluOpType.add)

            # g = m * (a - 1) + 1, staged into gbuf with a leading slot of ones
            gb = small.tile([N, TVp + V], f32)
            nc.vector.memset(gb[:, 0:V], 1.0)
            av3 = av.rearrange("n (t v) -> n t v", v=V)
            gb3 = _ap(gb, V, [gb.ap[0][:], [V, TP], [1, V]])
            nc.vector.scalar_tensor_tensor(
                out=gb3, in0=av3, scalar=1.0, in1=mb,
                op0=mybir.AluOpType.subtract, op1=mybir.AluOpType.mult)
            nc.vector.tensor_scalar(out=gb[:, V:V + TVp], in0=gb[:, V:V + TVp],
                                    scalar1=1.0, scalar2=None,
                                    op0=mybir.AluOpType.add)

            # exclusive cumulative product over t per part (scan on data0 =
            # [1, g_0, ..., g_{TP-2}])
            pb = small.tile([N, TVp], f32)
            for p in range(V):
                dview = _ap(gb, p, [gb.ap[0][:], [V, TP]])
                oview = _ap(pb, p, [pb.ap[0][:], [V, TP]])
                nc.vector.tensor_tensor_scan(
                    out=oview, data0=dview, data1=dview, initial=1.0,
                    op0=mybir.AluOpType.mult, op1=mybir.AluOpType.bypass)

            mc = small.tile([N, TP, V], f32)
            nc.vector.tensor_tensor(
                out=mc, in0=cc.rearrange("n (t v) -> n t v", v=V), in1=mb,
                op=mybir.AluOpType.mult)
            wco = small.tile([N, TVp], f32)
            nc.vector.tensor_tensor(out=wco, in0=mc.rearrange("n t v -> n (t v)"),
                                    in1=pb, op=mybir.AluOpType.mult)
            wco3 = wco.rearrange("n (t v) -> n t v", v=V)

            # ---- block-diagonal weights built on-chip ----
            # vis/mask rows are host-permuted to [evens | odds]; transpose
            # each parity block per part through a rotating pair of
            # bank-aligned PSUM tiles (HW transpose outputs must sit at
            # PSUM offset 0), draining each to SBUF via DVE.  The odd
            # parity block is then partition-shifted from base 0 to base
            # TP by one SBUF->SBUF DMA.
            wbd = small.tile([2 * TP, 2, NPAIR, V], bf16)
            nc.vector.memset(wbd, 0.0)
            sodd = small.tile([TP, NPAIR, V], bf16)
            for i in range(2):
                for p in range(V):
                    wtp = pst.tile([TP, 32], f32)
                    nc.tensor.transpose(
                        out=wtp,
                        in_=wco3[i * 32:(i + 1) * 32, :, p],
                        identity=idn[i * 32:(i + 1) * 32, :])
                    dst = wbd[0:TP, 0, :, p] if i == 0 else sodd[:, :, p]
                    nc.vector.tensor_copy(out=dst, in_=wtp)
            nc.sync.dma_start(out=wbd[TP:2 * TP, 1, :, :], in_=sodd[:, :, :])

            # ---- stage 1: neT[d, n] = sum_t w[n, t, p(d)] * embs[n, t, d] ----
            neT_ps = ps.tile([128, DC, N], f32)
            neT_sb = bigp.tile([128, DC, N], bf16)
            for g in range(NGRP):
                et = ets[g]
                for jj in range(2):
                    ip = 2 * g + jj
                    lhs_all = et[:, jj, :]
                    for dc in range(DC):
                        nc.tensor.matmul(
                            out=neT_ps[:, dc, 2 * ip:2 * ip + 2],
                            lhsT=lhs_all[:, dc * 128:(dc + 1) * 128],
                            rhs=wbd[:, :, ip, dc // 2],
                            start=True, stop=True)
                # drain this group's columns to SBUF (bf16 downcast)
                nc.vector.tensor_copy(
                    out=neT_sb[:, :, 4 * g:4 * g + 4],
                    in_=neT_ps[:, :, 4 * g:4 * g + 4])

            # ---- stage 2: tok[n, k] = nm[n]*b[k] + sum_d neT[d, n]*wt[d, k] ----
            tok_ps = ps.tile([N, TOK], f32)
            nc.tensor.matmul(out=tok_ps, lhsT=nmb_sb, rhs=bt_sb,
                             start=True, stop=False)
            for dc in range(DC):
                nc.tensor.matmul(
                    out=tok_ps,
                    lhsT=neT_sb[:, dc, :],
                    rhs=wt_sb[:, dc, :],
                    start=False, stop=(dc == DC - 1))

            tok_sb = small.tile([N, TOK], f32)
            nc.vector.tensor_copy(out=tok_sb, in_=tok_ps)
            nc.sync.dma_start(out=out_c[:, :], in_=tok_sb)

            if debug:
                nc.sync.dma_start(out=dbg_wco[:, :], in_=wco)
                wbd_f = small.tile([2 * TP, 2 * NPAIR * V], f32)
                nc.vector.tensor_copy(
                    out=wbd_f, in_=wbd.rearrange("t a b c -> t (a b c)"))
                nc.sync.dma_start(out=dbg_wbd[:, :], in_=wbd_f)
                neT_f = small.tile([128, DC * N], f32)
                nc.vector.tensor_copy(
                    out=neT_f, in_=neT_sb.rearrange("d a b -> d (a b)"))
                nc.sync.dma_start(out=dbg_neT[:, :], in_=neT_f)

    nc.compile()
    return nc


@functools.lru_cache(maxsize=2)
def _get_nc(TP):
    return build_nc(TP)


def _prep_in_maps(embs, vis, masks, W, b):
    masks = np.asarray(masks)
    L = masks.sum(axis=2)                      # [B, N]
    TP = max(1, int(L.max()))

    # stable argsort of ~mask puts valid timesteps first, in t order
    order = np.argsort(~masks, axis=2, kind="stable")[:, :, :TP]  # [B,N,TP]

    embs_bf = np.asarray(embs).astype(ml_dtypes.bfloat16)
    embs_cmp = np.take_along_axis(embs_bf, order[..., None], axis=2)
    vis_cmp = np.take_along_axis(np.asarray(vis, np.float32),
                                 order[..., None], axis=2)  # [B,N,TP,V]
    mask_cmp = (np.arange(TP)[None, None, :] < L[..., None]).astype(np.float32)

    # row permutation: evens first, odds second (for parity-block transposes)
    perm = np.concatenate([np.arange(0, N, 2), np.arange(1, N, 2)])

    wt2 = np.ascontiguousarray(
        W.T.astype(ml_dtypes.bfloat16).reshape(DC, 128, TOK)
        .transpose(1, 0, 2).reshape(128, DC * TOK))
    bt = np.ascontiguousarray(b.astype(ml_dtypes.bfloat16)[None, :])
    idn = np.concatenate([np.eye(32, dtype=np.float32)] * 2, axis=0)

    in_maps = []
    for c in range(B):
        # [N, TP, D] -> [NGRP, 2(j), 2(i), TP, D] -> [NGRP, 2(i), TP, 2(j), D]
        eg = np.ascontiguousarray(
            embs_cmp[c].reshape(NGRP, 2, 2, TP, D).transpose(0, 2, 3, 1, 4))
        nmb = (L[c] > 0).astype(ml_dtypes.bfloat16)[None, :]
        in_maps.append({
            "embs_c": eg,
            "vis_c": np.ascontiguousarray(vis_cmp[c][perm].reshape(N, TP * V)),
            "mask_c": np.ascontiguousarray(mask_cmp[c][perm]),
            "wt_c": wt2,
            "nmb_c": np.ascontiguousarray(nmb),
            "bt_c": bt,
            "idn_c": idn,
        })
    return TP, in_maps


def run(embs, vis, masks, W, b, **run_kwargs):
    TP, in_maps = _prep_in_maps(embs, vis, masks, W, b)
    nc = _get_nc(TP)
    res = run_bass_kernel_spmd(nc, in_maps, core_ids=list(range(B)),
                               **run_kwargs)
    out = np.stack([res.results[c]["out_c"] for c in range(B)], axis=0)
    return out, res


def kernel(embs, vis, masks, W, b):
    out, _ = run(embs, vis, masks, W, b)
    return out


# revision 18
# speedup vs baseline: 1.4598x; 1.1946x over previous
"""SmartLinearAppearance Trainium2 kernel.

Reference semantics (per (b, n) tracklet, reverse-time scan t = T-1 .. 0):
    xor  = (nv != 0) ^ (v_t != 0)
    prod = nv * v_t
    a_t  = prod * alpha + xor * nv          # per-part coefficient on state
    c_t  = prod * (1 - alpha) + xor * v_t   # per-part coefficient on input
    if m_t: ne = a_t[p] * ne + c_t[p] * e_t ; nv = max(nv, v_t)
    tok = where(any_t m, ne @ W.T + b, 0)

The recurrence is linear in embs given coefficients derived only from
(vis, masks), so it is reformulated as a single weighted reduction:
    ne[n, d] = sum_t w[n, t, p(d)] * embs[n, t, d]
    w = m * c * cumprod_{t' < t}(m ? a : 1)
Masked timesteps are exact no-ops of the recurrence, so valid timesteps
are compacted on the host (ragged -> padded to the global max valid
length TP) and embs is pre-cast to bf16, shrinking the HBM stream to
TP/T * 1/2 of the naive f32 read.

The per-tracklet coefficient chain runs on-device from the compacted
(vis, masks), with tracklet rows host-permuted to [evens | odds].  The
block-diagonal per-pair weight matrix is built on-chip: per part, the
two parity blocks are copied into a zeroed [64, 2*TP] staging tile in
block-diagonal form (partition-aligned copies at bases 0/32), and one
PE transpose yields the [2*TP, 64] rhs block at PSUM base 0, drained by
a single full-range DVE copy -- no DRAM round trip and no DMA on the
critical path.  Bias + final masking are folded into the stage-2 matmul
accumulation using a host-provided (any-mask) row.

Sharding: data-parallel over B across the 8 cores; the Linear weights
are replicated (pre-transposed/pre-tiled on the host).
"""

import sys

sys.path.insert(0, "/opt/trn_rl_repo")

import functools

import ml_dtypes
import numpy as np

import concourse.bacc as bacc
import concourse.bass as bass
import concourse.tile as tile
from concourse import mybir
from concourse.bass_utils import run_bass_kernel_spmd

B, N, T, D, V, TOK = 8, 64, 64, 1792, 7, 512
P = 7          # parts; F = D // P = 256
F = D // P
ALPHA = float(np.float32(0.9))
ONE_MINUS_ALPHA = float(np.float32(1.0) - np.float32(0.9))
NPAIR = N // 2           # 32 tracklet pairs per core
NGRP = 16                # embs DMA groups (2 pairs each)
DC = D // 128            # 14 d-chunks of 128

f32 = mybir.dt.float32
bf16 = mybir.dt.bfloat16


def _ap(t, offset_elems, dims):
    """Raw AP on a DRAM tensor/tile: dims = [[step, count], ...] in elements."""
    base = t[:] if hasattr(t, "shape") else t
    return bass.AP(tensor=base.tensor, offset=base.offset + offset_elems, ap=dims)


def build_nc(TP, debug=False):
    TVp = TP * V
    SH = [k for k in (1, 2, 4, 8, 16, 32) if k < TP]
    PAD = (SH[-1] if SH else 1) * V
    nc = bacc.Bacc()

    # host layout: [NGRP, 2(member), TP, 2(pair-in-group), D]
    embs_c = nc.dram_tensor("embs_c", [NGRP, 2, TP, 2, D], bf16,
                            kind="ExternalInput")
    vis_c = nc.dram_tensor("vis_c", [N, TVp], f32, kind="ExternalInput")
    mask_c = nc.dram_tensor("mask_c", [N, TP], f32, kind="ExternalInput")
    wt_c = nc.dram_tensor("wt_c", [128, DC * TOK], bf16, kind="ExternalInput")
    nmb_c = nc.dram_tensor("nmb_c", [1, N], bf16, kind="ExternalInput")
    bt_c = nc.dram_tensor("bt_c", [1, TOK], bf16, kind="ExternalInput")
    idn_c = nc.dram_tensor("idn_c", [64, 64], f32, kind="ExternalInput")
    out_c = nc.dram_tensor("out_c", [N, TOK], f32, kind="ExternalOutput")
    if debug:
        dbg_wco = nc.dram_tensor("dbg_wco", [N, TVp], f32, kind="ExternalOutput")
        dbg_wbd = nc.dram_tensor("dbg_wbd", [2 * TP, V * N], f32,
                                 kind="ExternalOutput")
        dbg_neT = nc.dram_tensor("dbg_neT", [128, DC * N], f32,
                                 kind="ExternalOutput")

    with tile.TileContext(nc) as tc:
        with (
            tc.tile_pool(name="small", bufs=1) as small,
            tc.tile_pool(name="big", bufs=1) as bigp,
            tc.tile_pool(name="embs", bufs=NGRP) as ep,
            tc.tile_pool(name="ps", bufs=1, space="PSUM") as ps,
            tc.tile_pool(name="pst", bufs=2, space="PSUM") as pst,
        ):
            # ---- all bulk DMA on the gpsimd ring, small inputs first so
            # their descriptors beat the embs flood into the queues ----
            vis = small.tile([N, TVp], f32)
            nc.gpsimd.dma_start(out=vis, in_=vis_c[:, :])
            msk = small.tile([N, TP], f32)
            nc.gpsimd.dma_start(out=msk, in_=mask_c[:, :])
            nmb_sb = small.tile([1, N], bf16)
            nc.sync.dma_start(out=nmb_sb, in_=nmb_c[:, :])
            bt_sb = small.tile([1, TOK], bf16)
            nc.sync.dma_start(out=bt_sb, in_=bt_c[:, :])
            idn = small.tile([64, 64], f32)
            nc.sync.dma_start(out=idn, in_=idn_c[:, :])

            # embs stream: NGRP groups of 2 pairs
            ets = []
            for g in range(NGRP):
                et = ep.tile([2 * TP, 2, D], bf16)
                nc.gpsimd.dma_start(
                    out=et[:, :, :],
                    in_=_ap(embs_c, g * 2 * TP * 2 * D,
                            [[TP * 2 * D, 2], [2 * D, TP], [1, 2 * D]]))
                ets.append(et)

            # wt after the embs groups (its data is needed last)
            wt_sb = bigp.tile([128, DC, TOK], bf16)
            nc.gpsimd.dma_start(
                out=wt_sb, in_=_ap(wt_c, 0, [[DC * TOK, 128], [1, DC * TOK]]))

            # mask broadcast view [N, TP, V] (step-0 inner dim)
            mb = bass.AP(tensor=msk.tensor, offset=msk.offset,
                         ap=[msk.ap[0][:], [1, TP], [0, V]])
            vis3 = vis.rearrange("n (t v) -> n t v", v=V)

            # ---- coefficient computation on [N, TP*V] ----
            mv = small.tile([N, TP, V], f32)
            nc.vector.tensor_tensor(out=mv, in0=vis3, in1=mb,
                                    op=mybir.AluOpType.mult)
            mvf = mv.rearrange("n t v -> n (t v)")

            # exclusive masked suffix max over t (log-doubling, zero pad)
            sA = small.tile([N, TVp + PAD], f32)
            sB = small.tile([N, TVp + PAD], f32)
            nc.vector.memset(sA, 0.0)
            nc.vector.memset(sB, 0.0)
            nc.vector.tensor_copy(out=sA[:, 0:TVp - V], in_=mvf[:, V:TVp])
            src, dst = sA, sB
            for k in SH:
                nc.vector.tensor_tensor(
                    out=dst[:, 0:TVp], in0=src[:, 0:TVp],
                    in1=src[:, k * V:k * V + TVp], op=mybir.AluOpType.max)
                src, dst = dst, src
            nv = src[:, 0:TVp]  # exclusive suffix max, [N, TP*V]

            n0 = small.tile([N, TVp], f32)
            nc.vector.tensor_scalar(out=n0, in0=nv, scalar1=0.0, scalar2=None,
                                    op0=mybir.AluOpType.is_gt)
            v0 = small.tile([N, TVp], f32)
            nc.vector.tensor_scalar(out=v0, in0=vis, scalar1=0.0, scalar2=None,
                                    op0=mybir.AluOpType.is_gt)
            xr = small.tile([N, TVp], f32)
            nc.vector.tensor_tensor(out=xr, in0=n0, in1=v0,
                                    op=mybir.AluOpType.not_equal)
            prod = small.tile([N, TVp], f32)
            nc.vector.tensor_tensor(out=prod, in0=nv, in1=vis,
                                    op=mybir.AluOpType.mult)
            xnv = small.tile([N, TVp], f32)
            nc.vector.tensor_tensor(out=xnv, in0=xr, in1=nv,
                                    op=mybir.AluOpType.mult)
            av = small.tile([N, TVp], f32)
            nc.vector.scalar_tensor_tensor(
                out=av, in0=prod, scalar=ALPHA, in1=xnv,
                op0=mybir.AluOpType.mult, op1=mybir.AluOpType.add)
            xv = small.tile([N, TVp], f32)
            nc.vector.tensor_tensor(out=xv, in0=xr, in1=vis,
                                    op=mybir.AluOpType.mult)
            cc = small.tile([N, TVp], f32)
            nc.vector.scalar_tensor_tensor(
                out=cc, in0=prod, scalar=ONE_MINUS_ALPHA, in1=xv,
                op0=mybir.AluOpType.mult, op1=mybir.AluOpType.add)

            # g = m * (a - 1) + 1, staged into gbuf with a leading slot of ones
            gb = small.tile([N, TVp + V], f32)
            nc.vector.memset(gb[:, 0:V], 1.0)
            av3 = av.rearrange("n (t v) -> n t v", v=V)
            gb3 = _ap(gb, V, [gb.ap[0][:], [V, TP], [1, V]])
            nc.vector.scalar_tensor_tensor(
                out=gb3, in0=av3, scalar=1.0, in1=mb,
                op0=mybir.AluOpType.subtract, op1=mybir.AluOpType.mult)
            nc.vector.tensor_scalar(out=gb[:, V:V + TVp], in0=gb[:, V:V + TVp],
                                    scalar1=1.0, scalar2=None,
                                    op0=mybir.AluOpType.add)

            # exclusive cumulative product over t per part (scan on data0 =
            # [1, g_0, ..., g_{TP-2}])
            pb = small.tile([N, TVp], f32)
            for p in range(V):
                dview = _ap(gb, p, [gb.ap[0][:], [V, TP]])
                oview = _ap(pb, p, [pb.ap[0][:], [V, TP]])
                nc.vector.tensor_tensor_scan(
                    out=oview, data0=dview, data1=dview, initial=1.0,
                    op0=mybir.AluOpType.mult, op1=mybir.AluOpType.bypass)

            mc = small.tile([N, TP, V], f32)
            nc.vector.tensor_tensor(
                out=mc, in0=cc.rearrange("n (t v) -> n t v", v=V), in1=mb,
                op=mybir.AluOpType.mult)
            wco = small.tile([N, TVp], f32)
            nc.vector.tensor_tensor(out=wco, in0=mc.rearrange("n t v -> n (t v)"),
                                    in1=pb, op=mybir.AluOpType.mult)
            wco3 = wco.rearrange("n (t v) -> n t v", v=V)

            # ---- block-diagonal weights built on-chip ----
            # Rows are host-permuted to [evens | odds].  Per part, copy the
            # two parity blocks into a zeroed [64, 2*TP] staging tile in
            # block-diagonal position (aligned partition bases 0 / 32), then
            # one PE transpose yields the [2*TP, 64] rhs block at PSUM base
            # 0, drained by one full-range DVE copy.
            in2a = small.tile([64, 2 * TP], f32)
            in2b = small.tile([64, 2 * TP], f32)
            nc.vector.memset(in2a, 0.0)
            nc.vector.memset(in2b, 0.0)
            in2 = [in2a, in2b]
            wbd = small.tile([2 * TP, V, N], bf16)
            for p in range(V):
                stg = in2[p % 2]
                nc.vector.tensor_copy(out=stg[0:32, 0:TP],
                                      in_=wco3[0:32, :, p])
                nc.vector.tensor_copy(out=stg[32:64, TP:2 * TP],
                                      in_=wco3[32:64, :, p])
                wtp = pst.tile([2 * TP, 64], f32)
                nc.tensor.transpose(out=wtp, in_=stg[:, :], identity=idn)
                nc.vector.tensor_copy(out=wbd[:, p, :], in_=wtp)

            # ---- stage 1: neT[d, n] = sum_t w[n, t, p(d)] * embs[n, t, d] ----
            # ping-pong PSUM tiles so group g+1's matmuls overlap group g's
            # PSUM->SBUF drain
            neT_pp0 = ps.tile([128, DC, 4], f32)
            neT_pp1 = ps.tile([128, DC, 4], f32)
            neT_pp = [neT_pp0, neT_pp1]
            neT_sb = bigp.tile([128, DC, N], bf16)
            for g in range(NGRP):
                et = ets[g]
                pp = neT_pp[g % 2]
                for jj in range(2):
                    ip = 2 * g + jj
                    lhs_all = et[:, jj, :]
                    for dc in range(DC):
                        # rhs cols: perm positions (ip, 32+ip) = old (2ip, 2ip+1)
                        nc.tensor.matmul(
                            out=pp[:, dc, 2 * jj:2 * jj + 2],
                            lhsT=lhs_all[:, dc * 128:(dc + 1) * 128],
                            rhs=_ap(wbd, (dc // 2) * N + ip,
                                    [wbd.ap[0][:], [32, 2]]),
                            start=True, stop=True)
                nc.vector.tensor_copy(
                    out=neT_sb[:, :, 4 * g:4 * g + 4],
                    in_=pp[:, :, 0:4])

            # ---- stage 2: tok[n, k] = nm[n]*b[k] + sum_d neT[d, n]*wt[d, k] ----
            tok_ps = ps.tile([N, TOK], f32)
            nc.tensor.matmul(out=tok_ps, lhsT=nmb_sb, rhs=bt_sb,
                             start=True, stop=False)
            for dc in range(DC):
                nc.tensor.matmul(
                    out=tok_ps,
                    lhsT=neT_sb[:, dc, :],
                    rhs=wt_sb[:, dc, :],
                    start=False, stop=(dc == DC - 1))

            tok_sb = small.tile([N, TOK], f32)
            nc.vector.tensor_copy(out=tok_sb, in_=tok_ps)
            nc.sync.dma_start(out=out_c[:, :], in_=tok_sb)

            if debug:
                nc.sync.dma_start(out=dbg_wco[:, :], in_=wco)
                wbd_f = small.tile([2 * TP, V * N], f32)
                nc.vector.tensor_copy(
                    out=wbd_f, in_=wbd.rearrange("t a b -> t (a b)"))
                nc.sync.dma_start(out=dbg_wbd[:, :], in_=wbd_f)
                neT_f = small.tile([128, DC * N], f32)
                nc.vector.tensor_copy(
                    out=neT_f, in_=neT_sb.rearrange("d a b -> d (a b)"))
                nc.sync.dma_start(out=dbg_neT[:, :], in_=neT_f)

    nc.compile()
    return nc


@functools.lru_cache(maxsize=2)
def _get_nc(TP):
    return build_nc(TP)


def _prep_in_maps(embs, vis, masks, W, b):
    masks = np.asarray(masks)
    L = masks.sum(axis=2)                      # [B, N]
    TP = max(1, int(L.max()))

    # stable argsort of ~mask puts valid timesteps first, in t order
    order = np.argsort(~masks, axis=2, kind="stable")[:, :, :TP]  # [B,N,TP]

    embs_bf = np.asarray(embs).astype(ml_dtypes.bfloat16)
    embs_cmp = np.take_along_axis(embs_bf, order[..., None], axis=2)
    vis_cmp = np.take_along_axis(np.asarray(vis, np.float32),
                                 order[..., None], axis=2)  # [B,N,TP,V]
    mask_cmp = (np.arange(TP)[None, None, :] < L[..., None]).astype(np.float32)

    # row permutation for the chain: evens first, odds second
    perm = np.concatenate([np.arange(0, N, 2), np.arange(1, N, 2)])

    wt2 = np.ascontiguousarray(
        W.T.astype(ml_dtypes.bfloat16).reshape(DC, 128, TOK)
        .transpose(1, 0, 2).reshape(128, DC * TOK))
    bt = np.ascontiguousarray(b.astype(ml_dtypes.bfloat16)[None, :])
    idn = np.eye(64, dtype=np.float32)

    in_maps = []
    for c in range(B):
        # [N, TP, D] -> [NGRP, 2(j), 2(i), TP, D] -> [NGRP, 2(i), TP, 2(j), D]
        eg = np.ascontiguousarray(
            embs_cmp[c].reshape(NGRP, 2, 2, TP, D).transpose(0, 2, 3, 1, 4))
        nmb = (L[c] > 0).astype(ml_dtypes.bfloat16)[None, :]
        in_maps.append({
            "embs_c": eg,
            "vis_c": np.ascontiguousarray(vis_cmp[c][perm].reshape(N, TP * V)),
            "mask_c": np.ascontiguousarray(mask_cmp[c][perm]),
            "wt_c": wt2,
            "nmb_c": np.ascontiguousarray(nmb),
            "bt_c": bt,
            "idn_c": idn,
        })
    return TP, in_maps


def run(embs, vis, masks, W, b, **run_kwargs):
    TP, in_maps = _prep_in_maps(embs, vis, masks, W, b)
    nc = _get_nc(TP)
    res = run_bass_kernel_spmd(nc, in_maps, core_ids=list(range(B)),
                               **run_kwargs)
    out = np.stack([res.results[c]["out_c"] for c in range(B)], axis=0)
    return out, res


def kernel(embs, vis, masks, W, b):
    out, _ = run(embs, vis, masks, W, b)
    return out
